# revision 1
# baseline (speedup 1.0000x reference)
"""KAN layer (Chebyshev order-7 on tanh(x)) as a Bass/Tile TRN2 kernel.

Math: out[b,o] = sum_{i,k} T_k(tanh(x[b,i])) * W[o,i,k] + bias[o],  k=0..7.

T_0 == 1, so the k=0 weight slice folds into an effective bias on the host:
bias_eff[o] = bias[o] + sum_i W[o,i,0]. The device contracts over the
remaining 7*1024 = 7168 (i,k) pairs.

Sharding: data-parallel over batch. Each of the 8 cores takes 512 batch
rows; every core holds the full weights. Per core this is a
[7168 x 512] basis (built on-chip from x) against [7168 x 1024] weights,
accumulated as out.T tiles [128(o) x 512(b)] across 8 PSUM banks with
fp32r matmuls (full PE rate at free-dim 512).
"""

import sys

sys.path.insert(0, "/opt/trn_rl_repo")

import numpy as np

import concourse.bass as bass  # noqa: F401  (engine types come via bacc)
import concourse.mybir as mybir
from concourse import bacc
from concourse.bass_utils import run_bass_kernel_spmd
from concourse.tile import TileContext

P = 128
N_CORES = 8
BATCH = 4096
B_CORE = BATCH // N_CORES  # 512
IN_F = 1024
OUT_F = 1024
KORD = 7  # Chebyshev T_1..T_7 (T_0 folded into bias)
N_ITILES = IN_F // P  # 8
N_OTILES = OUT_F // P  # 8
NSTEPS = N_ITILES * KORD  # 56 contraction steps of K=128

F32 = mybir.dt.float32
F32R = mybir.dt.float32r
ACT_COPY = mybir.ActivationFunctionType.Copy
ACT_TANH = mybir.ActivationFunctionType.Tanh
MULT = mybir.AluOpType.mult

_NC_CACHE = None


def _build():
    """Build + compile the single-core Bass program (SPMD across 8 cores)."""
    global _NC_CACHE
    if _NC_CACHE is not None:
        return _NC_CACHE

    nc = bacc.Bacc("TRN2", target_bir_lowering=False, debug=False)

    # xT[i, b] = x[b, i] for this core's batch slice.
    xT = nc.declare_dram_parameter("xT", [IN_F, B_CORE], F32, isOutput=False)
    # wT[it, k', p, o] = weights[o, it*128+p, k'+1]  (fp32 bits, fp32r view).
    wT = nc.declare_dram_parameter(
        "wT", [N_ITILES, KORD, P, OUT_F], F32R, isOutput=False
    )
    # biasT[p, ot] = bias_eff[ot*128 + p]
    biasT = nc.declare_dram_parameter("biasT", [P, N_OTILES], F32, isOutput=False)
    outT = nc.declare_dram_parameter("outT", [OUT_F, B_CORE], F32, isOutput=True)

    with TileContext(nc) as tc:
        with (
            tc.tile_pool(name="basis", bufs=1) as basis_pool,
            tc.tile_pool(name="chain", bufs=8) as chain_pool,
            tc.tile_pool(name="tmp", bufs=3) as tmp_pool,
            tc.tile_pool(name="raw", bufs=2) as raw_pool,
            tc.tile_pool(name="w", bufs=8) as w_pool,
            tc.tile_pool(name="osb", bufs=3) as osb_pool,
            tc.tile_pool(name="misc", bufs=1) as misc_pool,
            tc.tile_pool(name="psum", bufs=1, space="PSUM") as psum_pool,
        ):
            bias_sb = misc_pool.tile([P, N_OTILES], F32, name="bias_sb")
            nc.sync.dma_start(out=bias_sb, in_=biasT[:, :])

            # ---- Chebyshev basis: chain in fp32, fp32r copies for the PE ----
            # basis_r[it][j] = T_{j+1}(tanh(xT tile it)) as [128, 512] fp32r
            basis_r = []
            for it in range(N_ITILES):
                traw = raw_pool.tile([P, B_CORE], F32, tag="traw")
                nc.sync.dma_start(out=traw, in_=xT[it * P : (it + 1) * P, :])
                t = chain_pool.tile([P, B_CORE], F32, tag="chain")
                nc.scalar.activation(t, traw, ACT_TANH)

                tiles_r = []
                t1r = basis_pool.tile([P, B_CORE], F32R, name=f"b_{it}_0")
                nc.scalar.activation(t1r, t, ACT_COPY)
                tiles_r.append(t1r)

                prev, prev2 = t, None
                for k in range(2, KORD + 1):
                    tmp = tmp_pool.tile([P, B_CORE], F32, tag="tmp")
                    # tmp = (t * 2) * T_{k-1}
                    nc.vector.scalar_tensor_tensor(
                        out=tmp, in0=t, scalar=2.0, in1=prev, op0=MULT, op1=MULT
                    )
                    cur = chain_pool.tile([P, B_CORE], F32, tag="chain")
                    if k == 2:
                        nc.vector.tensor_scalar_sub(cur, tmp, 1.0)
                    else:
                        nc.vector.tensor_sub(cur, tmp, prev2)
                    ckr = basis_pool.tile([P, B_CORE], F32R, name=f"b_{it}_{k - 1}")
                    nc.scalar.activation(ckr, cur, ACT_COPY)
                    tiles_r.append(ckr)
                    prev2, prev = prev, cur
                basis_r.append(tiles_r)

            # ---- Matmul accumulation: out.T[ot] += w_s[:, ot].T @ basis_s ----
            psums = [
                psum_pool.tile([P, B_CORE], F32, name=f"ps_{ot}")
                for ot in range(N_OTILES)
            ]
            HALF = OUT_F // 2
            s = 0
            for it in range(N_ITILES):
                for k in range(KORD):
                    # split the weight fetch so the first 4 matmuls can
                    # start as soon as half the step's weights land
                    wa = w_pool.tile([P, HALF], F32R, tag="wa")
                    nc.sync.dma_start(out=wa, in_=wT[it, k, :, :HALF])
                    wb = w_pool.tile([P, HALF], F32R, tag="wb")
                    nc.sync.dma_start(out=wb, in_=wT[it, k, :, HALF:])
                    rhs = basis_r[it][k]
                    for ot in range(N_OTILES):
                        wt = wa if ot < 4 else wb
                        col = (ot % 4) * P
                        nc.tensor.matmul(
                            psums[ot],
                            lhsT=wt[:, col : col + P],
                            rhs=rhs,
                            start=(s == 0),
                            stop=(s == NSTEPS - 1),
                        )
                    s += 1

            # ---- bias add + store ----
            for ot in range(N_OTILES):
                osb = osb_pool.tile([P, B_CORE], F32, tag="osb")
                nc.scalar.activation(
                    osb,
                    psums[ot],
                    mybir.ActivationFunctionType.Identity,
                    bias=bias_sb[:, ot : ot + 1],
                    scale=1.0,
                )
                nc.sync.dma_start(out=outT[ot * P : (ot + 1) * P, :], in_=osb)

    nc.compile()
    _NC_CACHE = nc
    return _NC_CACHE


def _prep_inputs(x, weights, bias_param):
    x = np.asarray(x, dtype=np.float32)
    weights = np.asarray(weights, dtype=np.float32)
    bias_param = np.asarray(bias_param, dtype=np.float32)

    # [o, i, k] -> [it, k'=k-1, p, o], contiguous
    w4 = weights.transpose(1, 2, 0)[:, 1:, :]  # [i, 7, o]
    w4 = np.ascontiguousarray(
        w4.reshape(N_ITILES, P, KORD, OUT_F).transpose(0, 2, 1, 3)
    )

    bias_eff = bias_param + weights[:, :, 0].sum(axis=1)  # T_0 == 1 fold
    bias_t = np.ascontiguousarray(bias_eff.reshape(N_OTILES, P).T)  # [128, 8]

    in_maps = []
    for c in range(N_CORES):
        x_c = np.ascontiguousarray(x[c * B_CORE : (c + 1) * B_CORE].T)  # [1024, 512]
        in_maps.append({"xT": x_c, "wT": w4, "biasT": bias_t})
    return in_maps


def _run(x, weights, bias_param, **spmd_kwargs):
    nc = _build()
    in_maps = _prep_inputs(x, weights, bias_param)
    res = run_bass_kernel_spmd(nc, in_maps, core_ids=list(range(N_CORES)), **spmd_kwargs)
    out = np.empty((BATCH, OUT_F), dtype=np.float32)
    for c in range(N_CORES):
        out[c * B_CORE : (c + 1) * B_CORE] = res.results[c]["outT"].T
    return out, res


def kernel(x, weights, bias_param):
    out, _ = _run(x, weights, bias_param)
    return out



# revision 2
# speedup vs baseline: 1.5543x; 1.5543x over previous
"""KAN layer (Chebyshev order-7 on tanh(x)) as a Bass/Tile TRN2 kernel.

Math: out[b,o] = sum_{i,k} T_k(tanh(x[b,i])) * W[o,i,k] + bias[o],  k=0..7.

T_0 == 1 folds into an effective bias on the host. The device contracts
the remaining 7*1024 = 7168 (i,k) pairs per output.

Device strategy (data-parallel over batch, 512 rows/core):
- Basis is built on-chip in fp16: u = tanh(x), T_2 = 2u^2 - 1, then the
  even/odd Chebyshev recurrences T_{k+2} = (2 T_2) T_k - T_{k-2} as fp16
  tensor_tensor ops on the DVE (2x perf mode for 2-byte dtypes).
- The matmul runs in fp8e4 (e4m3) with DoubleRow perf mode at half a
  cycle per output row. The two DoubleRow "planes" carry a hi/lo split
  of the weights (Wh = fp8(W*2^12), Wl = fp8(W*2^12 - Wh)) against the
  same fp8 basis tile (stride-0 broadcast rhs), which cancels the
  weight-quantization error. Basis is quantized once (scale 64).
- psum accumulates in f32; output = psum * 2^-18 + bias_eff in fp16.
"""

import sys

sys.path.insert(0, "/opt/trn_rl_repo")

import math

import ml_dtypes
import numpy as np

import concourse.bass as bass  # noqa: F401  (engine types come via bacc)
import concourse.mybir as mybir
from concourse import bacc
from concourse.bass_utils import run_bass_kernel_spmd
from concourse.tile import TileContext

P = 128
N_CORES = 8
BATCH = 4096
B_CORE = BATCH // N_CORES  # 512
IN_F = 1024
OUT_F = 1024
KORD = 7  # Chebyshev T_1..T_7 (T_0 folded into bias)
N_ITILES = IN_F // P  # 8
N_OTILES = OUT_F // P  # 8
CHUNK = 2048  # free-dim chunk: 4 itiles per chunk
N_CHUNKS = 2
SB = 64.0  # basis fp8 scale
SW = 4096.0  # weight fp8 scale (2^12)
DESCALE = 1.0 / (SB * SW)

F32 = mybir.dt.float32
F16 = mybir.dt.float16
F8 = mybir.dt.float8e4
DR = mybir.MatmulPerfMode.DoubleRow
ACT_COPY = mybir.ActivationFunctionType.Copy
ACT_TANH = mybir.ActivationFunctionType.Tanh
ACT_SQUARE = mybir.ActivationFunctionType.Square
ACT_ID = mybir.ActivationFunctionType.Identity
MULT = mybir.AluOpType.mult
SUBTRACT = mybir.AluOpType.subtract
ADD = mybir.AluOpType.add

_NC_CACHE = None


def _build():
    """Build + compile the single-core Bass program (SPMD across 8 cores)."""
    global _NC_CACHE
    if _NC_CACHE is not None:
        return _NC_CACHE

    nc = bacc.Bacc("TRN2", target_bir_lowering=False, debug=False)

    # xT16[p, it*512 + b] = x[b, it*128 + p] as fp16, for this core's slice.
    xT16 = nc.declare_dram_parameter("xT16", [P, N_ITILES * B_CORE], F16, isOutput=False)
    # wT[k', it, p, ot*256 + pl*128 + o] = {Wh,Wl}[ot*128+o, it*128+p, k'+1]
    wT = nc.declare_dram_parameter(
        "wT", [KORD, N_ITILES, P, N_OTILES * 2 * P], F8, isOutput=False
    )
    # biasT[p, ot] = bias_eff[ot*128 + p]
    biasT = nc.declare_dram_parameter("biasT", [P, N_OTILES], F32, isOutput=False)
    # outT[ot, p, b] = out[b, ot*128 + p] fp16
    outT = nc.declare_dram_parameter("outT", [N_OTILES, P, B_CORE], F16, isOutput=True)

    with TileContext(nc) as tc:
        with (
            tc.tile_pool(name="x", bufs=1) as x_pool,
            tc.tile_pool(name="chain", bufs=1) as chain_pool,
            tc.tile_pool(name="tmp", bufs=4) as tmp_pool,
            tc.tile_pool(name="b8", bufs=1) as b8_pool,
            tc.tile_pool(name="w", bufs=10) as w_pool,
            tc.tile_pool(name="osb", bufs=3) as osb_pool,
            tc.tile_pool(name="misc", bufs=1) as misc_pool,
            tc.tile_pool(name="psum", bufs=1, space="PSUM") as psum_pool,
        ):
            bias_sb = misc_pool.tile([P, N_OTILES], F32, name="bias_sb")
            nc.sync.dma_start(out=bias_sb, in_=biasT[:, :])

            ones = misc_pool.tile([P, CHUNK], F16, name="ones")
            nc.vector.memset(ones, 1.0)

            xs = []
            for c in range(N_CHUNKS):
                xc = x_pool.tile([P, CHUNK], F16, name=f"x_{c}")
                nc.sync.dma_start(out=xc, in_=xT16[:, c * CHUNK : (c + 1) * CHUNK])
                xs.append(xc)

            # ---- Chebyshev basis in fp16; fp8 casts (scale 64) on Act ----
            # b8[k][c] holds fp8(T_k * 64) for itiles 4c..4c+3.
            b8 = [[None] * N_CHUNKS for _ in range(KORD + 1)]

            def cast(k, c, src):
                t = b8_pool.tile([P, CHUNK], F8, name=f"b8_{k}_{c}")
                nc.scalar.activation(t, src, ACT_COPY, scale=SB)
                b8[k][c] = t

            u, sq, T2, M = [], [], [], []
            for c in range(N_CHUNKS):
                uc = chain_pool.tile([P, CHUNK], F16, name=f"u_{c}")
                nc.scalar.activation(uc, xs[c], ACT_TANH)
                u.append(uc)
            for c in range(N_CHUNKS):
                cast(1, c, u[c])
            for c in range(N_CHUNKS):
                sqc = chain_pool.tile([P, CHUNK], F16, name=f"sq_{c}")
                nc.scalar.activation(sqc, u[c], ACT_SQUARE, scale=math.sqrt(2.0))
                sq.append(sqc)
            for c in range(N_CHUNKS):
                t2c = chain_pool.tile([P, CHUNK], F16, name=f"T2_{c}")
                nc.vector.tensor_tensor(out=t2c, in0=sq[c], in1=ones, op=SUBTRACT)
                T2.append(t2c)
            for c in range(N_CHUNKS):
                cast(2, c, T2[c])
            for c in range(N_CHUNKS):
                mc = chain_pool.tile([P, CHUNK], F16, name=f"M_{c}")
                nc.vector.tensor_tensor(out=mc, in0=T2[c], in1=T2[c], op=ADD)
                M.append(mc)

            # T_{k+2} = M * T_k - T_{k-2}; odd chain from (T_1), even from (T_2).
            Tk = {1: u, 2: T2}
            prev_of = {3: (1, 1), 4: (2, 0), 5: (3, 1), 6: (4, 2), 7: (5, 3)}
            for k in range(3, KORD + 1):
                src_k, sub_k = prev_of[k]
                cur = []
                for c in range(N_CHUNKS):
                    tm = tmp_pool.tile([P, CHUNK], F16, tag="tmp")
                    nc.vector.tensor_tensor(out=tm, in0=M[c], in1=Tk[src_k][c], op=MULT)
                    tk = chain_pool.tile([P, CHUNK], F16, name=f"T{k}_{c}")
                    sub_src = ones if sub_k == 0 else Tk[sub_k][c]
                    nc.vector.tensor_tensor(out=tk, in0=tm, in1=sub_src, op=SUBTRACT)
                    cur.append(tk)
                Tk[k] = cur
                for c in range(N_CHUNKS):
                    cast(k, c, cur[c])

            # ---- DoubleRow fp8 matmuls: psum[ot] += (Wh|Wl).T @ (Bh, Bh) ----
            psums = [
                psum_pool.tile([P, B_CORE], F32, name=f"ps_{ot}")
                for ot in range(N_OTILES)
            ]
            s = 0
            NSTEPS = KORD * N_ITILES
            for k in range(1, KORD + 1):
                for c in range(N_CHUNKS):
                    for itl in range(N_ITILES // N_CHUNKS):
                        it = c * (N_ITILES // N_CHUNKS) + itl
                        wsb = w_pool.tile([P, N_OTILES * 2 * P], F8, tag="w")
                        nc.sync.dma_start(out=wsb, in_=wT[k - 1, it, :, :])
                        rhs = (
                            b8[k][c][:, itl * B_CORE : (itl + 1) * B_CORE]
                            .unsqueeze(1)
                            .broadcast_to([P, 2, B_CORE])
                        )
                        for ot in range(N_OTILES):
                            lhsT = wsb[:, ot * 2 * P : (ot + 1) * 2 * P].rearrange(
                                "p (two m) -> p two m", two=2
                            )
                            nc.tensor.matmul(
                                psums[ot],
                                lhsT=lhsT,
                                rhs=rhs,
                                start=(s == 0),
                                stop=(s == NSTEPS - 1),
                                perf_mode=DR,
                            )
                        s += 1

            # ---- descale + bias add + store (fp16) ----
            for ot in range(N_OTILES):
                osb = osb_pool.tile([P, B_CORE], F16, tag="osb")
                nc.scalar.activation(
                    osb,
                    psums[ot],
                    ACT_ID,
                    bias=bias_sb[:, ot : ot + 1],
                    scale=DESCALE,
                )
                nc.sync.dma_start(out=outT[ot, :, :], in_=osb)

    nc.compile()
    _NC_CACHE = nc
    return _NC_CACHE


def _prep_inputs(x, weights, bias_param):
    x = np.asarray(x, dtype=np.float32)
    weights = np.asarray(weights, dtype=np.float32)
    bias_param = np.asarray(bias_param, dtype=np.float32)
    f8 = ml_dtypes.float8_e4m3

    # Weights: [o, i, k] -> hi/lo fp8 at scale 2^12, laid out
    # wT[k', it, p, ot, pl, o] with the last 3 dims contiguous (2KB lines).
    W7 = weights[:, :, 1:] * np.float32(SW)  # [o, i, 7]
    Wh = W7.astype(f8)
    Wl = (W7 - Wh.astype(np.float32)).astype(f8)
    arr = np.stack([Wh, Wl], axis=-1)  # [o_g, i_g, k, pl]
    arr = arr.reshape(N_OTILES, P, N_ITILES, P, KORD, 2)  # [ot, o, it, p, k, pl]
    wT = np.ascontiguousarray(arr.transpose(4, 2, 3, 0, 5, 1)).reshape(
        KORD, N_ITILES, P, N_OTILES * 2 * P
    )

    bias_eff = bias_param + weights[:, :, 0].sum(axis=1)  # T_0 == 1 fold
    bias_t = np.ascontiguousarray(bias_eff.reshape(N_OTILES, P).T)  # [128, 8]

    in_maps = []
    for cidx in range(N_CORES):
        xc = x[cidx * B_CORE : (cidx + 1) * B_CORE]  # [512, 1024]
        xt = np.ascontiguousarray(
            xc.T.reshape(N_ITILES, P, B_CORE).transpose(1, 0, 2).reshape(
                P, N_ITILES * B_CORE
            )
        ).astype(np.float16)
        in_maps.append({"xT16": xt, "wT": wT, "biasT": bias_t})
    return in_maps


def _run(x, weights, bias_param, **spmd_kwargs):
    nc = _build()
    in_maps = _prep_inputs(x, weights, bias_param)
    res = run_bass_kernel_spmd(nc, in_maps, core_ids=list(range(N_CORES)), **spmd_kwargs)
    out = np.empty((BATCH, OUT_F), dtype=np.float32)
    for cidx in range(N_CORES):
        o = res.results[cidx]["outT"]  # [8, 128, 512] fp16
        out[cidx * B_CORE : (cidx + 1) * B_CORE] = (
            np.asarray(o).astype(np.float32).transpose(2, 0, 1).reshape(B_CORE, OUT_F)
        )
    return out, res


def kernel(x, weights, bias_param):
    out, _ = _run(x, weights, bias_param)
    return out


# revision 6
# speedup vs baseline: 1.6815x; 1.0818x over previous
"""KAN layer (Chebyshev order-7 on tanh(x)) as a Bass/Tile TRN2 kernel.

Math: out[b,o] = sum_{i,k} T_k(tanh(x[b,i])) * W[o,i,k] + bias[o],  k=0..7.

T_0 == 1 folds into an effective bias on the host. The device contracts
the remaining 7*1024 = 7168 (i,k) pairs per output.

Device strategy (data-parallel over batch, 512 rows/core):
- Basis is built on-chip in fp16: u = tanh(x), T_2 = 2u^2 - 1, then the
  even/odd Chebyshev recurrences T_{k+2} = (2 T_2) T_k - T_{k-2} as fp16
  tensor_tensor ops on the DVE (2x perf mode for 2-byte dtypes).
- The matmul runs in fp8e4 (e4m3) with DoubleRow perf mode at half a
  cycle per output row. The two DoubleRow "planes" carry a hi/lo split
  of the weights (Wh = fp8(W*2^12), Wl = fp8(W*2^12 - Wh)) against the
  same fp8 basis tile (stride-0 broadcast rhs), which cancels the
  weight-quantization error. Basis is quantized once (scale 64).
- psum accumulates in f32; output = psum * 2^-18 + bias_eff in fp16.
"""

import sys

sys.path.insert(0, "/opt/trn_rl_repo")

import math

import ml_dtypes
import numpy as np

import concourse.bass as bass  # noqa: F401  (engine types come via bacc)
import concourse.mybir as mybir
from concourse import bacc
from concourse.bass_utils import run_bass_kernel_spmd
from concourse.tile import TileContext

P = 128
N_CORES = 8
BATCH = 4096
B_CORE = BATCH // N_CORES  # 512
IN_F = 1024
OUT_F = 1024
KORD = 7  # Chebyshev T_1..T_7 (T_0 folded into bias)
N_ITILES = IN_F // P  # 8
N_OTILES = OUT_F // P  # 8
CHUNK = 2048  # free-dim chunk: 4 itiles per chunk
N_CHUNKS = 2
SB = 64.0  # basis fp8 scale
SW = 4096.0  # weight fp8 scale (2^12)
DESCALE = 1.0 / (SB * SW)

F32 = mybir.dt.float32
F16 = mybir.dt.float16
F8 = mybir.dt.float8e4
DR = mybir.MatmulPerfMode.DoubleRow
ACT_COPY = mybir.ActivationFunctionType.Copy
ACT_TANH = mybir.ActivationFunctionType.Tanh
ACT_SQUARE = mybir.ActivationFunctionType.Square
ACT_ID = mybir.ActivationFunctionType.Identity
MULT = mybir.AluOpType.mult
SUBTRACT = mybir.AluOpType.subtract
ADD = mybir.AluOpType.add

_NC_CACHE = None


def _build():
    """Build + compile the single-core Bass program (SPMD across 8 cores)."""
    global _NC_CACHE
    if _NC_CACHE is not None:
        return _NC_CACHE

    nc = bacc.Bacc("TRN2", target_bir_lowering=False, debug=False)

    # xT16[p, it*512 + b] = x[b, it*128 + p] as fp16, for this core's slice.
    xT16 = nc.declare_dram_parameter("xT16", [P, N_ITILES * B_CORE], F16, isOutput=False)
    # wT[k', it, p, ot*256 + pl*128 + o] = {Wh,Wl}[ot*128+o, it*128+p, k'+1]
    wT = nc.declare_dram_parameter(
        "wT", [KORD, N_ITILES, P, N_OTILES * 2 * P], F8, isOutput=False
    )
    # biasT[p, ot] = bias_eff[ot*128 + p]
    biasT = nc.declare_dram_parameter("biasT", [P, N_OTILES], F32, isOutput=False)
    # outT[ot, p, b] = out[b, ot*128 + p] fp16
    outT = nc.declare_dram_parameter("outT", [N_OTILES, P, B_CORE], F16, isOutput=True)

    with TileContext(nc) as tc:
        with (
            tc.tile_pool(name="x", bufs=1) as x_pool,
            tc.tile_pool(name="chain", bufs=1) as chain_pool,
            tc.tile_pool(name="tmp", bufs=4) as tmp_pool,
            tc.tile_pool(name="b8", bufs=1) as b8_pool,
            tc.tile_pool(name="w", bufs=12) as w_pool,
            tc.tile_pool(name="osb", bufs=3) as osb_pool,
            tc.tile_pool(name="misc", bufs=1) as misc_pool,
            tc.tile_pool(name="psum", bufs=1, space="PSUM") as psum_pool,
        ):
            xs = []
            for c in range(N_CHUNKS):
                xc = x_pool.tile([P, CHUNK], F16, name=f"x_{c}")
                nc.sync.dma_start(out=xc, in_=xT16[:, c * CHUNK : (c + 1) * CHUNK])
                xs.append(xc)

            bias_sb = misc_pool.tile([P, N_OTILES], F32, name="bias_sb")
            nc.sync.dma_start(out=bias_sb, in_=biasT[:, :])

            ones = misc_pool.tile([P, CHUNK], F16, name="ones")
            nc.vector.memset(ones, 1.0)

            # ---- Chebyshev basis in fp16; fp8 casts (scale 64) on Act ----
            # b8[k][c] holds fp8(T_k * 64) for itiles 4c..4c+3.
            b8 = [[None] * N_CHUNKS for _ in range(KORD + 1)]

            def cast(k, c, src):
                t = b8_pool.tile([P, CHUNK], F8, name=f"b8_{k}_{c}")
                nc.scalar.activation(t, src, ACT_COPY, scale=SB)
                b8[k][c] = t

            # Act stream in consumption order: tanh_a, cast1_a first so the
            # PE can start on (k=1, chunk 0) as early as possible.
            u, sq, T2, M = [None] * N_CHUNKS, [None] * N_CHUNKS, [None] * N_CHUNKS, [None] * N_CHUNKS
            for c in range(N_CHUNKS):
                uc = chain_pool.tile([P, CHUNK], F16, name=f"u_{c}")
                nc.scalar.activation(uc, xs[c], ACT_TANH)
                u[c] = uc
                cast(1, c, uc)
            for c in range(N_CHUNKS):
                sqc = chain_pool.tile([P, CHUNK], F16, name=f"sq_{c}")
                nc.scalar.activation(sqc, u[c], ACT_SQUARE, scale=math.sqrt(2.0))
                sq[c] = sqc
                t2c = chain_pool.tile([P, CHUNK], F16, name=f"T2_{c}")
                nc.vector.tensor_tensor(out=t2c, in0=sqc, in1=ones, op=SUBTRACT)
                T2[c] = t2c
                cast(2, c, t2c)
                mc = chain_pool.tile([P, CHUNK], F16, name=f"M_{c}")
                nc.vector.tensor_tensor(out=mc, in0=t2c, in1=t2c, op=ADD)
                M[c] = mc

            # T_{k+2} = M * T_k - T_{k-2}; odd chain from (T_1), even from (T_2).
            Tk = {1: u, 2: T2}
            prev_of = {3: (1, 1), 4: (2, 0), 5: (3, 1), 6: (4, 2), 7: (5, 3)}
            for k in range(3, KORD + 1):
                src_k, sub_k = prev_of[k]
                cur = []
                for c in range(N_CHUNKS):
                    tm = tmp_pool.tile([P, CHUNK], F16, tag="tmp")
                    nc.vector.tensor_tensor(out=tm, in0=M[c], in1=Tk[src_k][c], op=MULT)
                    tk = chain_pool.tile([P, CHUNK], F16, name=f"T{k}_{c}")
                    sub_src = ones if sub_k == 0 else Tk[sub_k][c]
                    nc.vector.tensor_tensor(out=tk, in0=tm, in1=sub_src, op=SUBTRACT)
                    cur.append(tk)
                Tk[k] = cur
                for c in range(N_CHUNKS):
                    cast(k, c, cur[c])

            # ---- DoubleRow fp8 matmuls: psum[ot] += (Wh|Wl).T @ (Bh, Bh) ----
            psums = [
                psum_pool.tile([P, B_CORE], F32, name=f"ps_{ot}")
                for ot in range(N_OTILES)
            ]
            def rhs_for(k, c, itl):
                return (
                    b8[k][c][:, itl * B_CORE : (itl + 1) * B_CORE]
                    .unsqueeze(1)
                    .broadcast_to([P, 2, B_CORE])
                )

            def lhs_for(wsb, ot):
                return wsb[:, ot * 2 * P : (ot + 1) * 2 * P].rearrange(
                    "p (two m) -> p two m", two=2
                )

            ITL = N_ITILES // N_CHUNKS
            s = 0
            for k in range(1, KORD):
                for c in range(N_CHUNKS):
                    for itl in range(ITL):
                        it = c * ITL + itl
                        wsb = w_pool.tile([P, N_OTILES * 2 * P], F8, tag="w")
                        nc.sync.dma_start(out=wsb, in_=wT[k - 1, it, :, :])
                        rhs = rhs_for(k, c, itl)
                        for ot in range(N_OTILES):
                            nc.tensor.matmul(
                                psums[ot],
                                lhsT=lhs_for(wsb, ot),
                                rhs=rhs,
                                start=(s == 0),
                                stop=False,
                                perf_mode=DR,
                            )
                        s += 1

            # Last k-level runs ot-outer so each psum bank finishes early and
            # its descale+bias+store overlaps the remaining matmuls.
            w7 = []
            for it in range(N_ITILES):
                wsb = w_pool.tile([P, N_OTILES * 2 * P], F8, tag="w")
                nc.sync.dma_start(out=wsb, in_=wT[KORD - 1, it, :, :])
                w7.append(wsb)
            for ot in range(N_OTILES):
                for c in range(N_CHUNKS):
                    for itl in range(ITL):
                        it = c * ITL + itl
                        nc.tensor.matmul(
                            psums[ot],
                            lhsT=lhs_for(w7[it], ot),
                            rhs=rhs_for(KORD, c, itl),
                            start=False,
                            stop=(it == N_ITILES - 1),
                            perf_mode=DR,
                        )
                # ---- descale + bias add + store (fp16) ----
                osb = osb_pool.tile([P, B_CORE], F16, tag="osb")
                nc.scalar.activation(
                    osb,
                    psums[ot],
                    ACT_ID,
                    bias=bias_sb[:, ot : ot + 1],
                    scale=DESCALE,
                )
                nc.sync.dma_start(out=outT[ot, :, :], in_=osb)

    nc.compile()
    _NC_CACHE = nc
    return _NC_CACHE


def _prep_inputs(x, weights, bias_param):
    x = np.asarray(x, dtype=np.float32)
    weights = np.asarray(weights, dtype=np.float32)
    bias_param = np.asarray(bias_param, dtype=np.float32)
    f8 = ml_dtypes.float8_e4m3

    # Weights: [o, i, k] -> hi/lo fp8 at scale 2^12, laid out
    # wT[k', it, p, ot, pl, o] with the last 3 dims contiguous (2KB lines).
    W7 = weights[:, :, 1:] * np.float32(SW)  # [o, i, 7]
    Wh = W7.astype(f8)
    Wl = (W7 - Wh.astype(np.float32)).astype(f8)
    arr = np.stack([Wh, Wl], axis=-1)  # [o_g, i_g, k, pl]
    arr = arr.reshape(N_OTILES, P, N_ITILES, P, KORD, 2)  # [ot, o, it, p, k, pl]
    wT = np.ascontiguousarray(arr.transpose(4, 2, 3, 0, 5, 1)).reshape(
        KORD, N_ITILES, P, N_OTILES * 2 * P
    )

    bias_eff = bias_param + weights[:, :, 0].sum(axis=1)  # T_0 == 1 fold
    bias_t = np.ascontiguousarray(bias_eff.reshape(N_OTILES, P).T)  # [128, 8]

    in_maps = []
    for cidx in range(N_CORES):
        xc = x[cidx * B_CORE : (cidx + 1) * B_CORE]  # [512, 1024]
        xt = np.ascontiguousarray(
            xc.T.reshape(N_ITILES, P, B_CORE).transpose(1, 0, 2).reshape(
                P, N_ITILES * B_CORE
            )
        ).astype(np.float16)
        in_maps.append({"xT16": xt, "wT": wT, "biasT": bias_t})
    return in_maps


def _run(x, weights, bias_param, **spmd_kwargs):
    nc = _build()
    in_maps = _prep_inputs(x, weights, bias_param)
    res = run_bass_kernel_spmd(nc, in_maps, core_ids=list(range(N_CORES)), **spmd_kwargs)
    out = np.empty((BATCH, OUT_F), dtype=np.float32)
    for cidx in range(N_CORES):
        o = res.results[cidx]["outT"]  # [8, 128, 512] fp16
        out[cidx * B_CORE : (cidx + 1) * B_CORE] = (
            np.asarray(o).astype(np.float32).transpose(2, 0, 1).reshape(B_CORE, OUT_F)
        )
    return out, res


def kernel(x, weights, bias_param):
    out, _ = _run(x, weights, bias_param)
    return out


# revision 9
# speedup vs baseline: 1.8179x; 1.0811x over previous
"""KAN layer (Chebyshev order-7 on tanh(x)) as a Bass/Tile TRN2 kernel.

Math: out[b,o] = sum_{i,k} T_k(tanh(x[b,i])) * W[o,i,k] + bias[o],  k=0..7.

T_0 == 1 folds into an effective bias on the host. The device contracts
the remaining 7*1024 = 7168 (i,k) pairs per output.

Device strategy (data-parallel over batch, 512 rows/core):
- Basis is built on-chip in fp16: u = tanh(x), T_2 = 2u^2 - 1, then the
  even/odd Chebyshev recurrences T_{k+2} = (2 T_2) T_k - T_{k-2} as fp16
  tensor_tensor ops on the DVE (2x perf mode for 2-byte dtypes).
- The matmul runs in fp8e4 (e4m3) with DoubleRow perf mode at half a
  cycle per output row. The two DoubleRow "planes" carry a hi/lo split
  of the weights (Wh = fp8(W*2^12), Wl = fp8(W*2^12 - Wh)) against the
  same fp8 basis tile (stride-0 broadcast rhs), which cancels the
  weight-quantization error. Basis is quantized once (scale 64).
- psum accumulates in f32; output = psum * 2^-18 + bias_eff in fp16.
"""

import sys

sys.path.insert(0, "/opt/trn_rl_repo")

import math

import ml_dtypes
import numpy as np

import concourse.bass as bass  # noqa: F401  (engine types come via bacc)
import concourse.mybir as mybir
from concourse import bacc
from concourse.bass_utils import run_bass_kernel_spmd
from concourse.tile import TileContext

P = 128
N_CORES = 8
BATCH = 4096
B_CORE = BATCH // N_CORES  # 512
IN_F = 1024
OUT_F = 1024
KORD = 7  # Chebyshev T_1..T_7 (T_0 folded into bias)
N_ITILES = IN_F // P  # 8
N_OTILES = OUT_F // P  # 8
CHUNK = 2048  # free-dim chunk: 4 itiles per chunk
N_CHUNKS = 2
SB = 64.0  # basis fp8 scale
SW = 4096.0  # weight fp8 scale (2^12)
DESCALE = 1.0 / (SB * SW)

F32 = mybir.dt.float32
F16 = mybir.dt.float16
F8 = mybir.dt.float8e4
DR = mybir.MatmulPerfMode.DoubleRow
ACT_COPY = mybir.ActivationFunctionType.Copy
ACT_TANH = mybir.ActivationFunctionType.Tanh
ACT_SQUARE = mybir.ActivationFunctionType.Square
ACT_ID = mybir.ActivationFunctionType.Identity
MULT = mybir.AluOpType.mult
SUBTRACT = mybir.AluOpType.subtract
ADD = mybir.AluOpType.add

_NC_CACHE = None


def _build():
    """Build + compile the single-core Bass program (SPMD across 8 cores)."""
    global _NC_CACHE
    if _NC_CACHE is not None:
        return _NC_CACHE

    nc = bacc.Bacc("TRN2", target_bir_lowering=False, debug=False)

    # xT16[p, it*512 + b] = x[b, it*128 + p] as fp16, for this core's slice.
    xT16 = nc.declare_dram_parameter("xT16", [P, N_ITILES * B_CORE], F16, isOutput=False)
    # wT[k', it, p, ot*256 + pl*128 + o] = {Wh,Wl}[ot*128+o, it*128+p, k'+1]
    wT = nc.declare_dram_parameter(
        "wT", [KORD, N_ITILES, P, N_OTILES * 2 * P], F8, isOutput=False
    )
    # biasT[p, ot] = bias_eff[ot*128 + p]
    biasT = nc.declare_dram_parameter("biasT", [P, N_OTILES], F32, isOutput=False)
    # outT[ot, p, b] = out[b, ot*128 + p] fp16
    outT = nc.declare_dram_parameter("outT", [N_OTILES, P, B_CORE], F16, isOutput=True)

    with TileContext(nc) as tc:
        with (
            tc.tile_pool(name="x", bufs=1) as x_pool,
            tc.tile_pool(name="chain", bufs=1) as chain_pool,
            tc.tile_pool(name="tmp", bufs=4) as tmp_pool,
            tc.tile_pool(name="b8", bufs=1) as b8_pool,
            tc.tile_pool(name="w", bufs=12) as w_pool,
            tc.tile_pool(name="osb", bufs=3) as osb_pool,
            tc.tile_pool(name="misc", bufs=1) as misc_pool,
            tc.tile_pool(name="psum", bufs=1, space="PSUM") as psum_pool,
        ):
            # x arrives in 4 pieces of [128, 1024] so tanh/cast/matmul can
            # start early; the first k=1 weight tile is interleaved between
            # x pieces on the (serial) DMA bus.
            PIECE = 1024
            N_PIECES = 4
            xps = []
            w_first = None
            for j in range(N_PIECES):
                xp = x_pool.tile([P, PIECE], F16, name=f"x_{j}")
                nc.sync.dma_start(out=xp, in_=xT16[:, j * PIECE : (j + 1) * PIECE])
                xps.append(xp)
                if j == 1:
                    w_first = w_pool.tile([P, N_OTILES * 2 * P], F8, tag="w")
                    nc.sync.dma_start(out=w_first, in_=wT[0, 0, :, :])

            bias_sb = misc_pool.tile([P, N_OTILES], F32, name="bias_sb")
            nc.sync.dma_start(out=bias_sb, in_=biasT[:, :])

            ones = misc_pool.tile([P, CHUNK], F16, name="ones")
            nc.vector.memset(ones, 1.0)

            # ---- Chebyshev basis in fp16; fp8 casts (scale 64) on Act ----
            # b8[k][c] holds fp8(T_k * 64) for itiles 4c..4c+3.
            b8 = [[None] * N_CHUNKS for _ in range(KORD + 1)]

            def cast(k, c, src):
                t = b8_pool.tile([P, CHUNK], F8, name=f"b8_{k}_{c}")
                nc.scalar.activation(t, src, ACT_COPY, scale=SB)
                b8[k][c] = t

            # u and b8[1] live as full-width tiles written piecewise so the
            # Act stream can emit tanh_p0, cast1_p0 before tanh_p1 etc.
            u_full = chain_pool.tile([P, N_ITILES * B_CORE], F16, name="u_full")
            b8_1 = b8_pool.tile([P, N_ITILES * B_CORE], F8, name="b8_1")
            b8[1] = [
                b8_1[:, c * CHUNK : (c + 1) * CHUNK] for c in range(N_CHUNKS)
            ]
            for j in range(N_PIECES):
                sl = slice(j * PIECE, (j + 1) * PIECE)
                nc.scalar.activation(u_full[:, sl], xps[j], ACT_TANH)
                nc.scalar.activation(b8_1[:, sl], u_full[:, sl], ACT_COPY, scale=SB)

            u = [u_full[:, c * CHUNK : (c + 1) * CHUNK] for c in range(N_CHUNKS)]
            sq, T2, M = [None] * N_CHUNKS, [None] * N_CHUNKS, [None] * N_CHUNKS
            for c in range(N_CHUNKS):
                sqc = chain_pool.tile([P, CHUNK], F16, name=f"sq_{c}")
                nc.scalar.activation(sqc, u[c], ACT_SQUARE, scale=math.sqrt(2.0))
                sq[c] = sqc
                t2c = chain_pool.tile([P, CHUNK], F16, name=f"T2_{c}")
                nc.vector.tensor_tensor(out=t2c, in0=sqc, in1=ones, op=SUBTRACT)
                T2[c] = t2c
                cast(2, c, t2c)
                mc = chain_pool.tile([P, CHUNK], F16, name=f"M_{c}")
                nc.vector.tensor_tensor(out=mc, in0=t2c, in1=t2c, op=ADD)
                M[c] = mc

            # T_{k+2} = M * T_k - T_{k-2}; odd chain from (T_1), even from (T_2).
            Tk = {1: u, 2: T2}
            prev_of = {3: (1, 1), 4: (2, 0), 5: (3, 1), 6: (4, 2), 7: (5, 3)}
            for k in range(3, KORD + 1):
                src_k, sub_k = prev_of[k]
                cur = []
                for c in range(N_CHUNKS):
                    tm = tmp_pool.tile([P, CHUNK], F16, tag="tmp")
                    nc.vector.tensor_tensor(out=tm, in0=M[c], in1=Tk[src_k][c], op=MULT)
                    tk = chain_pool.tile([P, CHUNK], F16, name=f"T{k}_{c}")
                    sub_src = ones if sub_k == 0 else Tk[sub_k][c]
                    nc.vector.tensor_tensor(out=tk, in0=tm, in1=sub_src, op=SUBTRACT)
                    cur.append(tk)
                Tk[k] = cur
                for c in range(N_CHUNKS):
                    cast(k, c, cur[c])

            # ---- DoubleRow fp8 matmuls: psum[ot] += (Wh|Wl).T @ (Bh, Bh) ----
            psums = [
                psum_pool.tile([P, B_CORE], F32, name=f"ps_{ot}")
                for ot in range(N_OTILES)
            ]
            def rhs_for(k, c, itl):
                return (
                    b8[k][c][:, itl * B_CORE : (itl + 1) * B_CORE]
                    .unsqueeze(1)
                    .broadcast_to([P, 2, B_CORE])
                )

            def lhs_for(wsb, ot):
                return wsb[:, ot * 2 * P : (ot + 1) * 2 * P].rearrange(
                    "p (two m) -> p two m", two=2
                )

            ITL = N_ITILES // N_CHUNKS
            s = 0
            for k in range(1, KORD):
                for c in range(N_CHUNKS):
                    for itl in range(ITL):
                        it = c * ITL + itl
                        if k == 1 and it == 0:
                            wsb = w_first
                        else:
                            wsb = w_pool.tile([P, N_OTILES * 2 * P], F8, tag="w")
                            nc.sync.dma_start(out=wsb, in_=wT[k - 1, it, :, :])
                        rhs = rhs_for(k, c, itl)
                        for ot in range(N_OTILES):
                            nc.tensor.matmul(
                                psums[ot],
                                lhsT=lhs_for(wsb, ot),
                                rhs=rhs,
                                start=(s == 0),
                                stop=False,
                                perf_mode=DR,
                            )
                        s += 1

            # Last k-level runs ot-outer so each psum bank finishes early and
            # its descale+bias+store overlaps the remaining matmuls.
            w7 = []
            for it in range(N_ITILES):
                wsb = w_pool.tile([P, N_OTILES * 2 * P], F8, tag="w")
                nc.sync.dma_start(out=wsb, in_=wT[KORD - 1, it, :, :])
                w7.append(wsb)
            for ot in range(N_OTILES):
                for c in range(N_CHUNKS):
                    for itl in range(ITL):
                        it = c * ITL + itl
                        nc.tensor.matmul(
                            psums[ot],
                            lhsT=lhs_for(w7[it], ot),
                            rhs=rhs_for(KORD, c, itl),
                            start=False,
                            stop=(it == N_ITILES - 1),
                            perf_mode=DR,
                        )
                # ---- descale + bias add + store (fp16) ----
                osb = osb_pool.tile([P, B_CORE], F16, tag="osb")
                nc.scalar.activation(
                    osb,
                    psums[ot],
                    ACT_ID,
                    bias=bias_sb[:, ot : ot + 1],
                    scale=DESCALE,
                )
                nc.sync.dma_start(out=outT[ot, :, :], in_=osb)

    nc.compile()
    _NC_CACHE = nc
    return _NC_CACHE


def _prep_inputs(x, weights, bias_param):
    x = np.asarray(x, dtype=np.float32)
    weights = np.asarray(weights, dtype=np.float32)
    bias_param = np.asarray(bias_param, dtype=np.float32)
    f8 = ml_dtypes.float8_e4m3

    # Weights: [o, i, k] -> hi/lo fp8 at scale 2^12, laid out
    # wT[k', it, p, ot, pl, o] with the last 3 dims contiguous (2KB lines).
    W7 = weights[:, :, 1:] * np.float32(SW)  # [o, i, 7]
    Wh = W7.astype(f8)
    Wl = (W7 - Wh.astype(np.float32)).astype(f8)
    arr = np.stack([Wh, Wl], axis=-1)  # [o_g, i_g, k, pl]
    arr = arr.reshape(N_OTILES, P, N_ITILES, P, KORD, 2)  # [ot, o, it, p, k, pl]
    wT = np.ascontiguousarray(arr.transpose(4, 2, 3, 0, 5, 1)).reshape(
        KORD, N_ITILES, P, N_OTILES * 2 * P
    )

    bias_eff = bias_param + weights[:, :, 0].sum(axis=1)  # T_0 == 1 fold
    bias_t = np.ascontiguousarray(bias_eff.reshape(N_OTILES, P).T)  # [128, 8]

    in_maps = []
    for cidx in range(N_CORES):
        xc = x[cidx * B_CORE : (cidx + 1) * B_CORE]  # [512, 1024]
        xt = np.ascontiguousarray(
            xc.T.reshape(N_ITILES, P, B_CORE).transpose(1, 0, 2).reshape(
                P, N_ITILES * B_CORE
            )
        ).astype(np.float16)
        in_maps.append({"xT16": xt, "wT": wT, "biasT": bias_t})
    return in_maps


def _run(x, weights, bias_param, **spmd_kwargs):
    nc = _build()
    in_maps = _prep_inputs(x, weights, bias_param)
    res = run_bass_kernel_spmd(nc, in_maps, core_ids=list(range(N_CORES)), **spmd_kwargs)
    out = np.empty((BATCH, OUT_F), dtype=np.float32)
    for cidx in range(N_CORES):
        o = res.results[cidx]["outT"]  # [8, 128, 512] fp16
        out[cidx * B_CORE : (cidx + 1) * B_CORE] = (
            np.asarray(o).astype(np.float32).transpose(2, 0, 1).reshape(B_CORE, OUT_F)
        )
    return out, res


def kernel(x, weights, bias_param):
    out, _ = _run(x, weights, bias_param)
    return out
